# revision 27
# baseline (speedup 1.0000x reference)
"""Multi-head GAT layer for Trainium2 — 8 heads sharded across 8 NeuronCores.

Per head h (N=4096 nodes, F=64 features):
    ltg   = graph @ W[h]                          [N, F]
    s     = ltg @ a_src,  d = ltg @ a_dst         [N]
    E     = leaky_relu(s[:, None] + d[None, :], 0.2)
    Alpha = softmax(E, axis=-1)
    out   = Alpha @ ltg

Key algebraic trick used on-device: with z = s_i + d_j and
M_ij = [z >= 0],

    exp(leaky_relu(z)) = M_ij * e^{s_i} e^{d_j} + (1-M_ij) * e^{0.2 s_i} e^{0.2 d_j}

so the whole N x N softmax reduces to ONE 0/1 mask-generation pass
(DVE tensor_scalar is_ge) plus masked matmuls on the PE:

    num_i = u_i * (M @ (v .* ltg))_i + u2_i * (T2 - (M @ (v2 .* ltg)))_i
    den_i = u_i * (M @ v)_i          + u2_i * (t2 - (M @ v2)_i)
    out_i = num_i / den_i

with u = e^s, v = e^d, u2 = e^{0.2 s}, v2 = e^{0.2 d}, and T2/t2 the
full column sums of v2 .* [ltg | 1] (complement of the mask is handled
via total-minus-masked, using identical bf16 summands for exact
consistency).  The (1-M) path never materializes.

Heads are fully independent: core h computes head h; no collectives.
"""

import os
from contextlib import ExitStack

import numpy as np

N, F_IN, F, H = 4096, 64, 64, 8
P = 128
NB = N // P           # 32 node blocks
ISUP = 4              # i-blocks per PSUM super-block (4 banks of accumulators)
NSUP = NB // ISUP     # 8 super iterations
USE_LO = bool(int(os.environ.get("GAT_USE_LO", "0")))  # hi+lo bf16 split of rhs
RC = 260 if USE_LO else 130  # R columns per j-block
# fraction of mask-generation work routed to ScalarE as sigmoid(BIG*z)
# (saturates to exact 0/1 away from the kink; kink itself is continuous)
SIG_NUM = int(os.environ.get("GAT_SIG_NUM", "1"))
SIG_DEN = int(os.environ.get("GAT_SIG_DEN", "8"))
SIG_SCALE = 65536.0
# skip LDWEIGHTS on lo-matmuls (weights identical to the preceding hi-matmul)
SKIP_LDW = bool(int(os.environ.get("GAT_SKIP_LDW", "1")))
LDW_OPT = bool(int(os.environ.get("GAT_LDW_OPT", "0")))

_CACHE = {}


def _patch_ldw_opt():
    """Let walrus elide back-to-back LDWEIGHTS with identical weights
    (the hi/lo matmul pairs share their mask weights)."""
    if _CACHE.get("ldw_patched"):
        return
    _CACHE["ldw_patched"] = True
    import concourse.bass_utils as bu

    orig = bu.run_command

    def patched(argv, **kw):
        argv = [
            "--enable-ldw-opt=true" if a == "--enable-ldw-opt=false" else a
            for a in argv
        ]
        return orig(argv, **kw)

    bu.run_command = patched


def _build():
    import concourse.bass as bass  # noqa: F401
    import concourse.mybir as mybir
    import concourse.tile as tile
    from concourse import bacc

    dt = mybir.dt
    f32 = dt.float32
    bf16 = dt.bfloat16
    Alu = mybir.AluOpType
    Act = mybir.ActivationFunctionType

    nc = bacc.Bacc("TRN2", debug=False, num_devices=H)
    graph_d = nc.dram_tensor("graph", [N, F_IN], f32, kind="ExternalInput").ap()
    w_d = nc.dram_tensor("w", [F_IN, F], f32, kind="ExternalInput").ap()
    a_d = nc.dram_tensor("a", [2, F], f32, kind="ExternalInput").ap()
    out_d = nc.dram_tensor("out", [N, F], f32, kind="ExternalOutput").ap()

    ident_d = nc.inline_tensor(np.eye(P, dtype=np.float32), name="ident")

    with tile.TileContext(nc) as tc, ExitStack() as ctx:
        persist = ctx.enter_context(tc.tile_pool(name="persist", bufs=1))

        identity = persist.tile([P, P], f32)
        nc.sync.dma_start(identity[:], ident_d.ap())
        ones_row = persist.tile([1, P], f32)
        nc.vector.memset(ones_row[:], 1.0)
        ones_row_bf = persist.tile([1, P], bf16)
        nc.vector.memset(ones_row_bf[:], 1.0)
        ones_col_bf = persist.tile([P, 1], bf16)
        nc.vector.memset(ones_col_bf[:], 1.0)

        # fused [W | w_s | w_d] rhs for the per-block projection matmul
        wssd = persist.tile([F_IN, F + 2], f32)
        nc.sync.dma_start(wssd[:, 0:F], w_d[:])
        a2_sb = persist.tile([F, 2], f32)
        nc.sync.dma_start(a2_sb[:], a_d.rearrange("t k -> k t"))

        gT = persist.tile([F_IN, N], f32)           # graph^T
        ltg_all = persist.tile([P, F * NB], f32)    # ltg, j-major blocks
        sd_col = persist.tile([P, 2 * NB], f32)     # per block b: cols 2b=s, 2b+1=d
        negsd = persist.tile([P, 2 * NB], f32)
        dscaled = persist.tile([P, 2 * NB], f32)    # SIG_SCALE * sd_col
        uv1 = persist.tile([P, 2 * NB], f32)        # exp(s), exp(d)
        uv2 = persist.tile([P, 2 * NB], f32)        # exp(.2 s), exp(.2 d)
        sdrow = persist.tile([2, N], bf16)          # s, d as rows (bcast feed)
        s_rep = persist.tile([P, N], bf16)          # s broadcast down partitions
        r_all = persist.tile([P, RC * NB], bf16)    # [R1|R2|v|v2] (+lo) per block
        t2rep = persist.tile([P, 66], f32)          # T2 (64), Σv (1), t2 (1) bcast

        with tc.tile_pool(name="sps", bufs=3, space="PSUM") as sps, \
             tc.tile_pool(name="t2ps", bufs=1, space="PSUM") as t2ps, \
             tc.tile_pool(name="ssb", bufs=3) as ssb, \
             tc.tile_pool(name="gp", bufs=4) as gp:
            # W^T (for w_s/w_d), then w_sd = W^T.T @ a2 ... = W @ a2 per column
            wT_ps = sps.tile([F, F_IN], f32, tag="sps")
            nc.tensor.transpose(wT_ps[:], wssd[:, 0:F], identity[0:F_IN, 0:F_IN])
            wT_sb = ssb.tile([F, F_IN], f32)
            nc.vector.tensor_copy(wT_sb[:], wT_ps[:])
            wsd_ps = sps.tile([F_IN, 2], f32, tag="sps")
            nc.tensor.matmul(wsd_ps[:], wT_sb[:], a2_sb[:])
            nc.vector.tensor_copy(wssd[:, F:F + 2], wsd_ps[:])

            # Pipelined setup, 8-block groups: DMA -> PE transpose -> gT copy
            # -> projection -> s/d tables -> R blocks -> T2, so the first
            # masks/matmuls can start after ~1/4 of the setup.
            r_v = r_all.rearrange("p (b c) -> p b c", c=RC)
            uv1_v = uv1.rearrange("p (b c) -> p b c", c=2)
            uv2_v = uv2.rearrange("p (b c) -> p b c", c=2)
            t2_ps = t2ps.tile([1, 66], f32)
            n_acc = NB * (2 if USE_LO else 1)

            def do_group(g):
                cols = slice(16 * g, 16 * g + 16)
                nc.vector.tensor_scalar(negsd[:, cols], sd_col[:, cols], -1.0,
                                        None, op0=Alu.mult)
                if SIG_NUM:
                    nc.vector.tensor_scalar(dscaled[:, cols], sd_col[:, cols],
                                            SIG_SCALE, None, op0=Alu.mult)
                nc.scalar.activation(uv1[:, cols], sd_col[:, cols], Act.Exp)
                nc.scalar.activation(uv2[:, cols], sd_col[:, cols], Act.Exp,
                                     scale=0.2)
                for bb in range(8 * g, 8 * g + 8):
                    ltg_b = ltg_all[:, bb * F:(bb + 1) * F]
                    v_col = uv1[:, 2 * bb + 1:2 * bb + 2]
                    v2_col = uv2[:, 2 * bb + 1:2 * bb + 2]
                    r0 = RC * bb
                    if not USE_LO:
                        nc.vector.tensor_scalar(r_all[:, r0:r0 + F], ltg_b,
                                                v_col, None, op0=Alu.mult)
                        nc.vector.tensor_scalar(r_all[:, r0 + F:r0 + 2 * F],
                                                ltg_b, v2_col, None,
                                                op0=Alu.mult)
                    else:
                        r1f = ssb.tile([P, F], f32, tag="r1f", name="r1f")
                        r2f = ssb.tile([P, F], f32, tag="r2f", name="r2f")
                        nc.vector.tensor_scalar(r1f[:], ltg_b, v_col, None,
                                                op0=Alu.mult)
                        nc.vector.tensor_scalar(r2f[:], ltg_b, v2_col, None,
                                                op0=Alu.mult)
                        nc.scalar.copy(r_all[:, r0:r0 + F], r1f[:])
                        nc.scalar.copy(r_all[:, r0 + F:r0 + 2 * F], r2f[:])
                        nc.vector.tensor_tensor(r_all[:, r0 + 130:r0 + 130 + F],
                                                r1f[:], r_all[:, r0:r0 + F],
                                                op=Alu.subtract)
                        nc.vector.tensor_tensor(
                            r_all[:, r0 + 130 + F:r0 + 130 + 2 * F], r2f[:],
                            r_all[:, r0 + F:r0 + 2 * F], op=Alu.subtract)
                bsl = slice(8 * g, 8 * g + 8)
                nc.vector.tensor_copy(r_v[:, bsl, 128], uv1_v[:, bsl, 1])
                nc.vector.tensor_copy(r_v[:, bsl, 129], uv2_v[:, bsl, 1])
                if USE_LO:
                    nc.vector.tensor_tensor(r_v[:, bsl, 258], uv1_v[:, bsl, 1],
                                            r_v[:, bsl, 128], op=Alu.subtract)
                    nc.vector.tensor_tensor(r_v[:, bsl, 259], uv2_v[:, bsl, 1],
                                            r_v[:, bsl, 129], op=Alu.subtract)
                for bb in range(8 * g, 8 * g + 8):
                    r0 = RC * bb
                    k = bb * (2 if USE_LO else 1)
                    nc.tensor.matmul(t2_ps[:], ones_col_bf[:],
                                     r_all[:, r0 + F:r0 + 130],
                                     start=(k == 0), stop=(k == n_acc - 1))
                    if USE_LO:
                        nc.tensor.matmul(t2_ps[:], ones_col_bf[:],
                                         r_all[:, r0 + 130 + F:r0 + 260],
                                         start=False, stop=(k + 1 == n_acc - 1))

            for b in range(NB):
                g_sb = gp.tile([P, F_IN], f32)
                nc.sync.dma_start(g_sb[:], graph_d[b * P:(b + 1) * P, :])
                gT_ps = sps.tile([F_IN, P], f32, tag="sps")
                nc.tensor.transpose(gT_ps[:], g_sb[:], identity[:])
                nc.scalar.copy(gT[:, b * P:(b + 1) * P], gT_ps[:])
                prj_ps = sps.tile([P, F + 2], f32, tag="sps")
                nc.tensor.matmul(prj_ps[:], gT[:, b * P:(b + 1) * P], wssd[:])
                nc.scalar.copy(ltg_all[:, b * F:(b + 1) * F], prj_ps[:, 0:F])
                nc.scalar.copy(sd_col[:, 2 * b:2 * b + 2], prj_ps[:, F:F + 2])
                if b % 4 == 3:
                    # s,d row slice + partition-broadcast of s (bf16 matmuls)
                    c = b // 4
                    srow_ps = sps.tile([2, 512], f32, tag="sps")
                    nc.tensor.matmul(srow_ps[:], wssd[:, F:F + 2],
                                     gT[:, c * 512:(c + 1) * 512])
                    nc.scalar.copy(sdrow[:, c * 512:(c + 1) * 512], srow_ps[:])
                    bc_ps = sps.tile([P, 512], f32, tag="sps")
                    nc.tensor.matmul(bc_ps[:], ones_row_bf[:],
                                     sdrow[0:1, c * 512:(c + 1) * 512])
                    nc.scalar.copy(s_rep[:, c * 512:(c + 1) * 512], bc_ps[:])
                if b % 8 == 7:
                    do_group(b // 8)
            t2_sb = ssb.tile([1, 66], f32)
            nc.vector.tensor_copy(t2_sb[:], t2_ps[:])
            t2rep_ps = sps.tile([P, 66], f32, tag="sps")
            nc.tensor.matmul(t2rep_ps[:], ones_row[:], t2_sb[:])
            nc.scalar.copy(t2rep[:], t2rep_ps[:])

        # ---- main masked-matmul loop ----
        with tc.tile_pool(name="mask", bufs=3) as mp, \
             tc.tile_pool(name="acc", bufs=2, space="PSUM") as accp, \
             tc.tile_pool(name="ep", bufs=3) as ep:
            for sup in range(NSUP):
                i0 = sup * ISUP * P
                mtiles = []
                for b in range(NB):
                    mt = mp.tile([P, ISUP * P], bf16, tag=f"m{b}", name=f"mask{b}")
                    if (b % SIG_DEN) < SIG_NUM:
                        nc.scalar.activation(
                            mt[:], s_rep[:, i0:i0 + ISUP * P], Act.Sigmoid,
                            bias=dscaled[:, 2 * b + 1:2 * b + 2], scale=SIG_SCALE)
                    else:
                        nc.vector.tensor_scalar(
                            mt[:], s_rep[:, i0:i0 + ISUP * P],
                            negsd[:, 2 * b + 1:2 * b + 2], None, op0=Alu.is_ge)
                    mtiles.append(mt)

                acc = accp.tile([P, 512 * ISUP], f32)  # 4 banks; slice t at 512t
                for b in range(NB):
                    r0 = RC * b
                    for t in range(ISUP):
                        lhs = mtiles[b][:, t * P:(t + 1) * P]
                        nc.tensor.matmul(
                            acc[:, 512 * t:512 * t + RC], lhs,
                            r_all[:, r0:r0 + RC],
                            start=(b == 0), stop=(b == NB - 1))

                # epilogue: out_i = (u*A1 + u2*(T2-A2m)) / (u*b1 + u2*(t2-b2m))
                # with USE_LO, hi+lo column pairs are first summed (Ae).
                acc_v = acc.rearrange("p (t x) -> p t x", x=512)
                u_v = uv1.rearrange("p (b c) -> p b c", c=2)[:, sup * ISUP:(sup + 1) * ISUP, 0]
                u2_v = uv2.rearrange("p (b c) -> p b c", c=2)[:, sup * ISUP:(sup + 1) * ISUP, 0]
                if USE_LO:
                    # fold lo columns into hi: ae_all[t] = acc_hi + acc_lo
                    # (DVE can read only one PSUM operand -> bounce lo via SBUF)
                    ae_all = ep.tile([P, ISUP * 130], f32, tag="ae_all")
                    for t in range(ISUP):
                        lo_sb = ep.tile([P, 130], f32, tag="lo_sb")
                        nc.scalar.copy(lo_sb[:],
                                       acc[:, 512 * t + 130:512 * t + 260])
                        nc.vector.tensor_tensor(
                            ae_all[:, 130 * t:130 * (t + 1)],
                            acc[:, 512 * t:512 * t + 130], lo_sb[:], op=Alu.add)
                    ae_v = ae_all.rearrange("p (t x) -> p t x", x=130)
                    b1v, b2v = ae_v[:, :, 128], ae_v[:, :, 129]
                else:
                    b1v, b2v = acc_v[:, :, 128], acc_v[:, :, 129]
                den1 = ep.tile([P, ISUP], f32)
                nc.vector.tensor_tensor(den1[:], u_v, b1v, op=Alu.mult)
                dd = ep.tile([P, ISUP], f32)
                nc.vector.tensor_tensor(dd[:], t2rep[:, 65:66].to_broadcast([P, ISUP]),
                                        b2v, op=Alu.subtract)
                den2 = ep.tile([P, ISUP], f32)
                nc.vector.tensor_tensor(den2[:], dd[:], u2_v, op=Alu.mult)
                den = ep.tile([P, ISUP], f32)
                nc.vector.tensor_tensor(den[:], den2[:], den1[:], op=Alu.add)
                rden = ep.tile([P, ISUP], f32)
                nc.vector.reciprocal(rden[:], den[:])

                for t in range(ISUP):
                    i = sup * ISUP + t
                    u_col = uv1[:, 2 * i:2 * i + 1]
                    u2_col = uv2[:, 2 * i:2 * i + 1]
                    if USE_LO:
                        a1 = ae_all[:, 130 * t:130 * t + F]
                        a2m = ae_all[:, 130 * t + F:130 * t + 2 * F]
                    else:
                        a1 = acc[:, 512 * t:512 * t + F]
                        a2m = acc[:, 512 * t + F:512 * t + 2 * F]
                    n1 = ep.tile([P, F], f32, tag="n1")
                    nc.scalar.mul(n1[:], a1, u_col)
                    d2 = ep.tile([P, F], f32, tag="d2")
                    nc.vector.tensor_tensor(d2[:], t2rep[:, 0:F], a2m,
                                            op=Alu.subtract)
                    n2 = ep.tile([P, F], f32, tag="n2")
                    nc.vector.tensor_scalar(n2[:], d2[:], u2_col, None, op0=Alu.mult)
                    num = ep.tile([P, F], f32, tag="num")
                    nc.vector.tensor_tensor(num[:], n1[:], n2[:], op=Alu.add)
                    ot = ep.tile([P, F], f32, tag="ot")
                    nc.scalar.mul(ot[:], num[:], rden[:, t:t + 1])
                    nc.sync.dma_start(out_d[i * P:(i + 1) * P, :], ot[:])

    if LDW_OPT:
        # walrus's ldw-opt refuses explicit InstLdweights; skip the bacc pass
        # that creates them (generate_event_semaphores covers multi-waits).
        nc.move_matmul_waits_to_ldweights = lambda: None
    nc.compile()
    return nc


def _get_nc():
    if "nc" not in _CACHE:
        _CACHE["nc"] = _build()
    return _CACHE["nc"]


def kernel(graph, W, a):
    from concourse.bass_utils import run_bass_kernel_spmd

    if LDW_OPT:
        _patch_ldw_opt()
    graph = np.ascontiguousarray(np.asarray(graph, dtype=np.float32))
    W = np.asarray(W, dtype=np.float32)
    a = np.asarray(a, dtype=np.float32)

    nc = _get_nc()
    in_maps = [
        {
            "graph": graph,
            "w": np.ascontiguousarray(W[h]),
            "a": np.ascontiguousarray(a[h].reshape(2, F)),
        }
        for h in range(H)
    ]
    trace = bool(int(os.environ.get("GAT_TRACE", "0")))
    res = run_bass_kernel_spmd(nc, in_maps, core_ids=list(range(H)), trace=trace)
    _CACHE["last_result"] = res
    return np.stack([res.results[h]["out"] for h in range(H)], axis=0)
